# revision 12
# baseline (speedup 1.0000x reference)
"""v3.2: (32,16) bin-split histogram, compares spread over DVE+Pool+Act.

Binning: y = bf16(2x + 195.5) -- the bf16 downcast rounds to nearest, giving
y = q + 192 exactly (q = round(2x + 3.5), floor-of-bin away from measure-zero
edges).

Codes (1/16-grid, exactly representable in bf16, with guard slots so
out-of-range coordinate values cannot alias a valid bin until |x| ~ 6):
  hi = q0 + 0.0625*(floor(q1/2) + 2)   -> 32 targets (a>>2) + 0.0625*((a&3)+2)
  lo = (q1 mod 2) + 0.0625*(q2 + 2)    -> 16 targets (b>>3) + 0.0625*((b&7)+2)

Stage-1 per batch: weights = 32-wide one-hot (Ldweights engine-free),
stream = 16-wide one-hot -> PSUM (32,16) accumulated over 64 j-columns.
Stage-2: 32 matmuls contract counts with host-split W-tiles into (40, nb).

Engine split: DVE does the full 32-wide one-hot (tensor_tensor is_equal, 2x
mode) + bf16 rounding chain (tensor_scalar 4x); Pool does 12 of the 16
lo-bins (per-bin tensor_scalar is_equal); Act does x->y affine prep and 4
lo-bins via Square + Relu(1-256*s) two-pass exact indicator.
"""

import numpy as np

B, N, VR, CLS = 1024, 8192, 8, 40
NCORES = 8
BPC = B // NCORES
PJ = 64
GV = 8           # batches per arith/pool/act instruction group
GO = 4           # batches per DVE one-hot instruction group
GD = 8           # batches per x DMA load
JW = GV * PJ     # free extent of arith group
JO = GO * PJ     # free extent of one-hot group
NPOOL = 12       # lo-bins on Pool
NACT = 4         # lo-bins on Act (two-pass) -- NPOOL + NACT <= 16

_CACHE = {}


def _t16v(bb):
    return float(bb >> 3) + 0.0625 * float((bb & 7) + 2)


def _build(nb):
    import concourse.bacc as bacc
    import concourse.mybir as mybir
    import concourse.tile as tile

    dt = mybir.dt
    op = mybir.AluOpType
    AF = mybir.ActivationFunctionType
    nc = bacc.Bacc("TRN2", target_bir_lowering=False, debug=False,
                   num_devices=NCORES)

    x_d = nc.dram_tensor("x", (128, 3, nb, PJ), dt.float32,
                         kind="ExternalInput")
    t32_d = nc.dram_tensor("t32", (1, 32 * JO), dt.bfloat16,
                           kind="ExternalInput")
    t16_d = nc.dram_tensor("t16", (1, 16 * JO), dt.bfloat16,
                           kind="ExternalInput")
    w2_d = nc.dram_tensor("w2", (32, 2, 16, CLS), dt.bfloat16,
                          kind="ExternalInput")
    bias_d = nc.dram_tensor("bias", (CLS, 1), dt.float32,
                            kind="ExternalInput")
    actb_d = nc.dram_tensor("actb", (1, max(NACT, 1)), dt.float32,
                            kind="ExternalInput")
    y_d = nc.dram_tensor("y", (CLS, nb), dt.float32, kind="ExternalOutput")

    NDVE = 16 - NPOOL - NACT  # leftover lo-bins on DVE

    with tile.TileContext(nc) as tc:
        with (
            tc.tile_pool(name="const", bufs=1) as cpool,
            tc.tile_pool(name="xg", bufs=2) as xpool,
            tc.tile_pool(name="code", bufs=4) as cdpool,
            tc.tile_pool(name="oh32", bufs=3) as oh32pool,
            tc.tile_pool(name="oh16", bufs=2) as oh16pool,
            tc.tile_pool(name="sq", bufs=2) as sqpool,
            tc.tile_pool(name="cnt", bufs=1) as cntpool,
            tc.tile_pool(name="ps1", bufs=2, space="PSUM") as ps1pool,
            tc.tile_pool(name="ps2", bufs=1, space="PSUM") as ps2pool,
        ):
            xg0 = xpool.tile([128, 3, GD, PJ], dt.float32, tag="xg")
            nc.sync.dma_start(xg0[:], x_d[:, :, 0:GD])
            t32 = cpool.tile([128, 32, JO], dt.bfloat16)
            nc.sync.dma_start(
                t32[:], t32_d.ap().broadcast_to((128, 32 * JO)).rearrange(
                    "p (a j) -> p a j", a=32))
            if NDVE > 0:
                t16 = cpool.tile([128, 16, JO], dt.bfloat16)
                nc.sync.dma_start(
                    t16[:], t16_d.ap().broadcast_to((128, 16 * JO)).rearrange(
                        "p (a j) -> p a j", a=16))
            w2 = cpool.tile([32, 2, 16, CLS], dt.bfloat16)
            nc.sync.dma_start(w2[:], w2_d[:])
            bias = cpool.tile([CLS, 1], dt.float32)
            nc.sync.dma_start(bias[:], bias_d[:])
            actb = cpool.tile([128, max(NACT, 1)], dt.float32)
            nc.sync.dma_start(
                actb[:], actb_d.ap().broadcast_to((128, max(NACT, 1))))

            cnt = cntpool.tile([32, nb, 16], dt.bfloat16)

            def _stage2(blo, bhi):
                ps2 = ps2pool.tile([CLS, bhi - blo], dt.float32,
                                   tag=f"ps2_{blo}")
                for h in range(2):
                    for m in range(16):
                        nc.tensor.matmul(ps2[:], w2[:, h, m],
                                         cnt[:, blo:bhi, m],
                                         start=(h == 0 and m == 0),
                                         stop=(h == 1 and m == 15))
                outt = cpool.tile([CLS, bhi - blo], dt.float32,
                                  tag=f"out_{blo}")
                nc.vector.tensor_scalar(outt[:], ps2[:], 1.0 / N, bias[:],
                                        op.mult, op.add)
                nc.sync.dma_start(y_d[:, blo:bhi], outt[:])

            for g in range(nb // GV):
                b0 = g * GV
                if b0 % GD == 0:
                    if b0 == 0:
                        xg = xg0
                    else:
                        xg = xpool.tile([128, 3, GD, PJ], dt.float32,
                                        tag="xg")
                        nc.sync.dma_start(xg[:], x_d[:, :, b0:b0 + GD])
                xs = xg[:, :, (b0 % GD):(b0 % GD) + GV].rearrange(
                    "p c g j -> p c (g j)")                  # (128,3,JW)

                y = cdpool.tile([128, 3, JW], dt.bfloat16, tag="y")
                nc.scalar.activation(y[:], xs, AF.Copy, bias=195.5, scale=2.0)
                y0, y1, y2 = y[:, 0], y[:, 1], y[:, 2]

                # lo chain first so Pool/Act one-hots can start early
                z1 = cdpool.tile([128, JW], dt.bfloat16, tag="z1")
                nc.vector.tensor_scalar(z1[:], y1, 0.5, 95.75,
                                        op.mult, op.add)
                u2p = cdpool.tile([128, JW], dt.bfloat16, tag="u2p")
                nc.vector.tensor_scalar(u2p[:], z1[:], 2.0, -192.0,
                                        op.mult, op.add)
                q2e = cdpool.tile([128, JW], dt.bfloat16, tag="q2e")
                nc.scalar.activation(q2e[:], y2, AF.Copy, bias=-11.875,
                                     scale=0.0625)
                r1 = cdpool.tile([128, JW], dt.bfloat16, tag="r1")
                nc.vector.tensor_tensor(r1[:], y1, u2p[:], op.subtract)
                lo = cdpool.tile([128, JW], dt.bfloat16, tag="lo")
                nc.vector.tensor_tensor(lo[:], r1[:], q2e[:], op.add)

                # hi chain
                q0t = cdpool.tile([128, JW], dt.bfloat16, tag="q0t")
                nc.scalar.activation(q0t[:], y0, AF.Copy, bias=-192.0,
                                     scale=1.0)
                z1q = cdpool.tile([128, JW], dt.bfloat16, tag="z1q")
                nc.scalar.activation(z1q[:], z1[:], AF.Copy, bias=-11.875,
                                     scale=0.0625)
                hi = cdpool.tile([128, JW], dt.bfloat16, tag="hi")
                nc.vector.tensor_tensor(hi[:], q0t[:], z1q[:], op.add)

                # one-hots (Pool/Act bins at GV granularity, DVE at GO)
                oh16 = oh16pool.tile([128, 16, JW], dt.bfloat16, tag="oh16")
                for bb in range(NPOOL):
                    nc.gpsimd.tensor_scalar(oh16[:, bb], lo[:], _t16v(bb),
                                            None, op.is_equal)
                sq = sqpool.tile([128, NACT, JW], dt.bfloat16, tag="sq")
                for i in range(NACT):
                    bb = NPOOL + i
                    nc.scalar.activation(sq[:, i], lo[:], AF.Square,
                                         bias=actb[:, i:i + 1], scale=1.0)
                    nc.scalar.activation(oh16[:, bb], sq[:, i], AF.Relu,
                                         bias=1.0, scale=-256.0)
                for h in range(JW // JO):
                    sl = slice(h * JO, (h + 1) * JO)
                    if NDVE > 0:
                        nc.vector.tensor_tensor(
                            oh16[:, 16 - NDVE:16, sl],
                            lo[:, sl].unsqueeze(1).broadcast_to(
                                (128, NDVE, JO)),
                            t16[:, 16 - NDVE:16], op.is_equal)
                    oh32 = oh32pool.tile([128, 32, JO], dt.bfloat16, tag="oh32")
                    nc.vector.tensor_tensor(
                        oh32[:],
                        hi[:, sl].unsqueeze(1).broadcast_to((128, 32, JO)),
                        t32[:], op.is_equal)

                    ps1 = ps1pool.tile([32, GO, 16], dt.float32, tag="ps1")
                    for v in range(GO):
                        for j in range(PJ):
                            t = v * PJ + j
                            nc.tensor.matmul(
                                ps1[:, v], oh32[:, :, t],
                                oh16[:, :, h * JO + t],
                                start=(j == 0), stop=(j == PJ - 1))
                    bi0 = b0 + h * GO
                    nc.scalar.copy(cnt[:, bi0:bi0 + GO, :], ps1[:])

                if b0 + GV == nb // 2:
                    _stage2(0, nb // 2)

            _stage2(nb // 2, nb)

    nc.compile()
    return nc


def _aux_inputs(W, b):
    from ml_dtypes import bfloat16 as bf16
    aa = np.arange(32)
    t32v = ((aa >> 2) + 0.0625 * ((aa & 3) + 2)).astype(np.float32)
    t32 = np.repeat(t32v, JO).astype(bf16).reshape(1, 32 * JO)
    bv = np.arange(16)
    t16v = ((bv >> 3) + 0.0625 * ((bv & 7) + 2)).astype(np.float32)
    t16 = np.repeat(t16v, JO).astype(bf16).reshape(1, 16 * JO)
    # w2[a, h, b, c] = hi/lo bf16 split of W[c, lin(a, b)]
    a = np.arange(32)[:, None]
    bb = np.arange(16)[None, :]
    lin = 64 * (a >> 2) + 8 * (2 * (a & 3) + (bb >> 3)) + (bb & 7)  # (32,16)
    wt = np.asarray(W, dtype=np.float32)[:, lin]          # (CLS, 32, 16)
    wt = np.ascontiguousarray(wt.transpose(1, 2, 0))      # (32, 16, CLS)
    whi = wt.astype(bf16)
    wlo = (wt - whi.astype(np.float32)).astype(bf16)
    w2 = np.ascontiguousarray(np.stack([whi, wlo], axis=1))  # (32,2,16,CLS)
    bias = np.asarray(b, dtype=np.float32).reshape(CLS, 1)
    actb = np.array([[-_t16v(NPOOL + i) for i in range(max(NACT, 1))]],
                    dtype=np.float32)
    return t32, t16, w2, bias, actb


def kernel(x, W, b):
    from concourse.bass_utils import run_bass_kernel_spmd

    x = np.asarray(x, dtype=np.float32)
    W = np.asarray(W, dtype=np.float32)
    b = np.asarray(b, dtype=np.float32)

    if BPC not in _CACHE:
        _CACHE[BPC] = _build(BPC)
    nc = _CACHE[BPC]

    t32, t16, w2, bias, actb = _aux_inputs(W, b)
    # (core, batch, p, j, coord) -> (core, p, coord, batch, j)
    shards = x.reshape(NCORES, BPC, 128, PJ, 3).transpose(0, 2, 4, 1, 3)
    in_maps = [
        {"x": np.ascontiguousarray(shards[i]), "t32": t32, "t16": t16,
         "w2": w2, "bias": bias, "actb": actb}
        for i in range(NCORES)
    ]
    res = run_bass_kernel_spmd(nc, in_maps, list(range(NCORES)))
    return np.concatenate(
        [np.asarray(res.results[i]["y"]).T for i in range(NCORES)],
        axis=0).astype(np.float32)


# revision 16
# speedup vs baseline: 1.3136x; 1.3136x over previous
"""v3.2: (32,16) bin-split histogram, compares spread over DVE+Pool+Act.

Binning: y = bf16(2x + 195.5) -- the bf16 downcast rounds to nearest, giving
y = q + 192 exactly (q = round(2x + 3.5), floor-of-bin away from measure-zero
edges).

Codes (1/16-grid, exactly representable in bf16, with guard slots so
out-of-range coordinate values cannot alias a valid bin until |x| ~ 6):
  hi = q0 + 0.0625*(floor(q1/2) + 2)   -> 32 targets (a>>2) + 0.0625*((a&3)+2)
  lo = (q1 mod 2) + 0.0625*(q2 + 2)    -> 16 targets (b>>3) + 0.0625*((b&7)+2)

Stage-1 per batch: weights = 32-wide one-hot (Ldweights engine-free),
stream = 16-wide one-hot -> PSUM (32,16) accumulated over 64 j-columns.
Stage-2: 32 matmuls contract counts with host-split W-tiles into (40, nb).

Engine split: DVE does the full 32-wide one-hot (tensor_tensor is_equal, 2x
mode) + bf16 rounding chain (tensor_scalar 4x); Pool does 12 of the 16
lo-bins (per-bin tensor_scalar is_equal); Act does x->y affine prep and 4
lo-bins via Square + Relu(1-256*s) two-pass exact indicator.
"""

import numpy as np

B, N, VR, CLS = 1024, 8192, 8, 40
NCORES = 8
BPC = B // NCORES
PJ = 64
GV = 8           # batches per arith/pool/act instruction group
OH32B = 2        # oh32 pool bufs
OH16B = 2        # oh16 pool bufs
CDB = 4          # code pool bufs
GO = 4           # batches per DVE one-hot instruction group
GD = 8           # batches per x DMA load
JW = GV * PJ     # free extent of arith group
JO = GO * PJ     # free extent of one-hot group
NPOOL = 9        # lo-bins on Pool
NACT = 4         # lo-bins on Act (two-pass) -- NPOOL + NACT <= 16
Q0T_ENG = 'dve'  # q0t on act|dve
Z1Q_ENG = 'act'  # z1q on act|dve
Q2E_ENG = 'pool'  # q2e on act|dve|pool

_CACHE = {}


def _t16v(bb):
    return float(bb >> 3) + 0.0625 * float((bb & 7) + 2)


def _build(nb):
    import concourse.bacc as bacc
    import concourse.mybir as mybir
    import concourse.tile as tile

    dt = mybir.dt
    op = mybir.AluOpType
    AF = mybir.ActivationFunctionType
    nc = bacc.Bacc("TRN2", target_bir_lowering=False, debug=False,
                   num_devices=NCORES)

    x_d = nc.dram_tensor("x", (128, 3, nb, PJ), dt.float32,
                         kind="ExternalInput")
    t32_d = nc.dram_tensor("t32", (1, 32 * JO), dt.bfloat16,
                           kind="ExternalInput")
    t16_d = nc.dram_tensor("t16", (1, 16 * JO), dt.bfloat16,
                           kind="ExternalInput")
    w2_d = nc.dram_tensor("w2", (32, 2, 16, CLS), dt.bfloat16,
                          kind="ExternalInput")
    bias_d = nc.dram_tensor("bias", (CLS, 1), dt.float32,
                            kind="ExternalInput")
    actb_d = nc.dram_tensor("actb", (1, max(NACT, 1)), dt.float32,
                            kind="ExternalInput")
    y_d = nc.dram_tensor("y", (CLS, nb), dt.float32, kind="ExternalOutput")

    NDVE = 16 - NPOOL - NACT  # leftover lo-bins on DVE

    with tile.TileContext(nc) as tc:
        with (
            tc.tile_pool(name="const", bufs=1) as cpool,
            tc.tile_pool(name="xg", bufs=2) as xpool,
            tc.tile_pool(name="code", bufs=CDB) as cdpool,
            tc.tile_pool(name="oh32", bufs=OH32B) as oh32pool,
            tc.tile_pool(name="oh16", bufs=OH16B) as oh16pool,
            tc.tile_pool(name="sq", bufs=2) as sqpool,
            tc.tile_pool(name="cnt", bufs=1) as cntpool,
            tc.tile_pool(name="ps1", bufs=2, space="PSUM") as ps1pool,
            tc.tile_pool(name="ps2", bufs=1, space="PSUM") as ps2pool,
        ):
            xg0 = xpool.tile([128, 3, GD, PJ], dt.float32, tag="xg")
            nc.sync.dma_start(xg0[:], x_d[:, :, 0:GD])
            w2 = cpool.tile([32, 2, 16, CLS], dt.bfloat16)
            nc.sync.dma_start(w2[:], w2_d[:])
            bias = cpool.tile([CLS, 1], dt.float32)
            nc.sync.dma_start(bias[:], bias_d[:])
            actb = cpool.tile([128, max(NACT, 1)], dt.float32)
            nc.sync.dma_start(
                actb[:], actb_d.ap().broadcast_to((128, max(NACT, 1))))

            cnt = cntpool.tile([32, nb, 16], dt.bfloat16)

            def _stage2(blo, bhi):
                ps2 = ps2pool.tile([CLS, bhi - blo], dt.float32,
                                   tag=f"ps2_{blo}")
                for h in range(2):
                    for m in range(16):
                        nc.tensor.matmul(ps2[:], w2[:, h, m],
                                         cnt[:, blo:bhi, m],
                                         start=(h == 0 and m == 0),
                                         stop=(h == 1 and m == 15))
                outt = cpool.tile([CLS, bhi - blo], dt.float32,
                                  tag=f"out_{blo}")
                nc.vector.tensor_scalar(outt[:], ps2[:], 1.0 / N, bias[:],
                                        op.mult, op.add)
                nc.sync.dma_start(y_d[:, blo:bhi], outt[:])

            for g in range(nb // GV):
                b0 = g * GV
                if b0 % GD == 0:
                    if b0 == 0:
                        xg = xg0
                    else:
                        xg = xpool.tile([128, 3, GD, PJ], dt.float32,
                                        tag="xg")
                        nc.sync.dma_start(xg[:], x_d[:, :, b0:b0 + GD])
                xs = xg[:, :, (b0 % GD):(b0 % GD) + GV].rearrange(
                    "p c g j -> p c (g j)")                  # (128,3,JW)

                y = cdpool.tile([128, 3, JW], dt.bfloat16, tag="y")
                nc.scalar.activation(y[:], xs, AF.Copy, bias=195.5, scale=2.0)
                y0, y1, y2 = y[:, 0], y[:, 1], y[:, 2]

                # lo chain first so Pool/Act one-hots can start early
                z1 = cdpool.tile([128, JW], dt.bfloat16, tag="z1")
                nc.vector.tensor_scalar(z1[:], y1, 0.5, 95.75,
                                        op.mult, op.add)
                u2p = cdpool.tile([128, JW], dt.bfloat16, tag="u2p")
                nc.vector.tensor_scalar(u2p[:], z1[:], 2.0, -192.0,
                                        op.mult, op.add)
                q2e = cdpool.tile([128, JW], dt.bfloat16, tag="q2e")
                if Q2E_ENG == 'act':
                    nc.scalar.activation(q2e[:], y2, AF.Copy, bias=-11.875,
                                         scale=0.0625)
                elif Q2E_ENG == 'pool':
                    nc.gpsimd.tensor_scalar(q2e[:], y2, 0.0625, -11.875,
                                            op.mult, op.add)
                else:
                    nc.vector.tensor_scalar(q2e[:], y2, 0.0625, -11.875,
                                            op.mult, op.add)
                r1 = cdpool.tile([128, JW], dt.bfloat16, tag="r1")
                nc.vector.tensor_tensor(r1[:], y1, u2p[:], op.subtract)
                lo = cdpool.tile([128, JW], dt.bfloat16, tag="lo")
                nc.vector.tensor_tensor(lo[:], r1[:], q2e[:], op.add)

                # hi chain
                q0t = cdpool.tile([128, JW], dt.bfloat16, tag="q0t")
                if Q0T_ENG == 'act':
                    nc.scalar.activation(q0t[:], y0, AF.Copy, bias=-192.0,
                                         scale=1.0)
                else:
                    nc.vector.tensor_scalar(q0t[:], y0, -192.0, None, op.add)
                z1q = cdpool.tile([128, JW], dt.bfloat16, tag="z1q")
                if Z1Q_ENG == 'act':
                    nc.scalar.activation(z1q[:], z1[:], AF.Copy, bias=-11.875,
                                         scale=0.0625)
                else:
                    nc.vector.tensor_scalar(z1q[:], z1[:], 0.0625, -11.875,
                                            op.mult, op.add)
                hi = cdpool.tile([128, JW], dt.bfloat16, tag="hi")
                nc.vector.tensor_tensor(hi[:], q0t[:], z1q[:], op.add)

                # one-hots: per-bin tensor_scalar is_equal (4x mode on DVE)
                oh16 = oh16pool.tile([128, 16, JW], dt.bfloat16, tag="oh16")
                for bb in range(NPOOL):
                    nc.gpsimd.tensor_scalar(oh16[:, bb], lo[:], _t16v(bb),
                                            None, op.is_equal)
                sq = sqpool.tile([128, NACT, JW], dt.bfloat16, tag="sq")
                for i in range(NACT):
                    bb = NPOOL + i
                    nc.scalar.activation(sq[:, i], lo[:], AF.Square,
                                         bias=actb[:, i:i + 1], scale=1.0)
                    nc.scalar.activation(oh16[:, bb], sq[:, i], AF.Relu,
                                         bias=1.0, scale=-256.0)
                for bb in range(NPOOL + NACT, 16):
                    nc.vector.tensor_scalar(oh16[:, bb], lo[:], _t16v(bb),
                                            None, op.is_equal)
                oh32 = oh32pool.tile([128, 32, JW], dt.bfloat16, tag="oh32")
                for aa2 in range(32):
                    tv = float(aa2 >> 2) + 0.0625 * float((aa2 & 3) + 2)
                    nc.vector.tensor_scalar(oh32[:, aa2], hi[:], tv,
                                            None, op.is_equal)

                for h in range(JW // JO):
                    ps1 = ps1pool.tile([32, GO, 16], dt.float32, tag="ps1")
                    for v in range(GO):
                        for j in range(PJ):
                            t = h * JO + v * PJ + j
                            nc.tensor.matmul(
                                ps1[:, v], oh32[:, :, t],
                                oh16[:, :, t],
                                start=(j == 0), stop=(j == PJ - 1))
                    bi0 = b0 + h * GO
                    nc.scalar.copy(cnt[:, bi0:bi0 + GO, :], ps1[:])

                if b0 + GV == nb // 2:
                    _stage2(0, nb // 2)

            _stage2(nb // 2, nb)

    nc.compile()
    return nc


def _aux_inputs(W, b):
    from ml_dtypes import bfloat16 as bf16
    aa = np.arange(32)
    t32v = ((aa >> 2) + 0.0625 * ((aa & 3) + 2)).astype(np.float32)
    t32 = np.repeat(t32v, JO).astype(bf16).reshape(1, 32 * JO)
    bv = np.arange(16)
    t16v = ((bv >> 3) + 0.0625 * ((bv & 7) + 2)).astype(np.float32)
    t16 = np.repeat(t16v, JO).astype(bf16).reshape(1, 16 * JO)
    # w2[a, h, b, c] = hi/lo bf16 split of W[c, lin(a, b)]
    a = np.arange(32)[:, None]
    bb = np.arange(16)[None, :]
    lin = 64 * (a >> 2) + 8 * (2 * (a & 3) + (bb >> 3)) + (bb & 7)  # (32,16)
    wt = np.asarray(W, dtype=np.float32)[:, lin]          # (CLS, 32, 16)
    wt = np.ascontiguousarray(wt.transpose(1, 2, 0))      # (32, 16, CLS)
    whi = wt.astype(bf16)
    wlo = (wt - whi.astype(np.float32)).astype(bf16)
    w2 = np.ascontiguousarray(np.stack([whi, wlo], axis=1))  # (32,2,16,CLS)
    bias = np.asarray(b, dtype=np.float32).reshape(CLS, 1)
    actb = np.array([[-_t16v(NPOOL + i) for i in range(max(NACT, 1))]],
                    dtype=np.float32)
    return t32, t16, w2, bias, actb


def kernel(x, W, b):
    from concourse.bass_utils import run_bass_kernel_spmd

    x = np.asarray(x, dtype=np.float32)
    W = np.asarray(W, dtype=np.float32)
    b = np.asarray(b, dtype=np.float32)

    if BPC not in _CACHE:
        _CACHE[BPC] = _build(BPC)
    nc = _CACHE[BPC]

    t32, t16, w2, bias, actb = _aux_inputs(W, b)
    # (core, batch, p, j, coord) -> (core, p, coord, batch, j)
    shards = x.reshape(NCORES, BPC, 128, PJ, 3).transpose(0, 2, 4, 1, 3)
    in_maps = [
        {"x": np.ascontiguousarray(shards[i]), "t32": t32, "t16": t16,
         "w2": w2, "bias": bias, "actb": actb}
        for i in range(NCORES)
    ]
    res = run_bass_kernel_spmd(nc, in_maps, list(range(NCORES)))
    return np.concatenate(
        [np.asarray(res.results[i]["y"]).T for i in range(NCORES)],
        axis=0).astype(np.float32)
